# revision 1
# baseline (speedup 1.0000x reference)
import sys

sys.path.insert(0, "/opt/trn_rl_repo")

import numpy as np
import concourse.bass as bass  # noqa: F401  (registers types)
from concourse import bacc
import concourse.mybir as mybir
from concourse.tile import TileContext
from concourse.bass_utils import run_bass_kernel_spmd

S = 4096          # sequence length
D = 1024          # model/key/value dim
NCORES = 8
R = S // NCORES   # 512 rows per core
KC = D // 128     # 8 contraction chunks
J = S // 128      # 32 key tiles
VA = D + 2        # V augmented with ones column (denominator) + zero pad (fp32r even-size rule)
CH = [(0, 342), (342, 342), (684, 342)]  # PV output column chunks (<=512 moving, >=256, even)
JQ = 8            # key tiles per PV quarter

F32 = mybir.dt.float32
F32R = mybir.dt.float32r

_cache = {}


def _build_phase1():
    """Per core: q = xs@(Wq/sqrt(D)), k = xs@Wk, v = xs@Wv for its 512-row x slice.

    One weight-load of each x chunk feeds 6 matmuls (3 projections x 2 column
    halves). Biases added on host.
      xsT [128, KC*R]: [p, k*R+i] = x[i, 128k+p]
      wq/wk/wv [128, KC*D]: [p, k*D+d] = W[128k+p, d]
    Outputs: q/k/v [R, D] natural layout.
    """
    nc = bacc.Bacc(None, target_bir_lowering=False)
    xsT = nc.dram_tensor("xsT", [128, KC * R], F32R, kind="ExternalInput")
    wins = [nc.dram_tensor(n, [128, KC * D], F32R, kind="ExternalInput")
            for n in ("wq", "wk", "wv")]
    outs = [nc.dram_tensor(n, [R, D], F32, kind="ExternalOutput") for n in ("q", "k", "v")]
    with TileContext(nc) as tc:
        with tc.tile_pool(name="inp", bufs=1) as inp, \
             tc.tile_pool(name="ob", bufs=6) as ob, \
             tc.tile_pool(name="ps", bufs=6, space="PSUM") as ps:
            xt = inp.tile([128, KC * R], F32R)
            wts = [inp.tile([128, KC * D], F32R, name=f"w{w_i}") for w_i in range(3)]
            # k-interleaved issue order: first matmul needs only the first two DMAs
            for k in range(KC):
                nc.sync.dma_start(xt[:, k * R : (k + 1) * R], xsT[:, k * R : (k + 1) * R])
                nc.sync.dma_start(wts[0][:, k * D : (k + 1) * D], wins[0][:, k * D : (k + 1) * D])
            for w_i in (1, 2):
                for k in range(KC):
                    nc.sync.dma_start(wts[w_i][:, k * D : (k + 1) * D], wins[w_i][:, k * D : (k + 1) * D])
            for w_i in range(3):
                for i in range(R // 128):
                    pz = [ps.tile([128, 512], F32, name=f"p{w_i}_{i}_{n2}", tag="ps")
                          for n2 in range(2)]
                    for k in range(KC):
                        lhsT = xt[:, k * R + i * 128 : k * R + i * 128 + 128]
                        for n2 in range(2):
                            nc.tensor.matmul(
                                pz[n2][:],
                                lhsT,
                                wts[w_i][:, k * D + n2 * 512 : k * D + (n2 + 1) * 512],
                                start=(k == 0), stop=(k == KC - 1),
                            )
                    for n2 in range(2):
                        o = ob.tile([128, 512], F32, name=f"o{w_i}_{i}_{n2}", tag="ob")
                        nc.vector.tensor_copy(o[:], pz[n2][:])
                        nc.sync.dma_start(
                            outs[w_i][i * 128 : (i + 1) * 128, n2 * 512 : (n2 + 1) * 512], o[:]
                        )
    nc.finalize()
    return nc


def _build_phase2():
    """Per core: anti-causal attention for its 512 query rows vs all 4096 keys.

    Scores computed transposed (S^T[j,i], keys on partitions), masked+exp'd via
    an iota<=thr data mask. P^T @ V_aug accumulates over j in PSUM per quarter
    (8 j-tiles), with one P^T weight-load per (i, j) feeding 3 column chunks.
    The ones column of V_aug yields the softmax denominator.
      qt [128, KC*R]: [p, k*R+i] = qT[128k+p, i]   (q pre-scaled by 1/sqrt(D))
      kt [128, J*D]:  [p, j*D + k*128 + c] = kT[128k+p, 128j+c]
      vi [128, J*VA]: [p, j*VA + c] = v_aug[128j+p, c]
      io [128, R]: iota row (0..R-1), th [128, J]: thr[p,j] = 128j+p-512*core
    Output rd [R, D] = normalized attention read.
    """
    nc = bacc.Bacc(None, target_bir_lowering=False)
    qt_in = nc.dram_tensor("qt", [128, KC * R], F32R, kind="ExternalInput")
    kt_in = nc.dram_tensor("kt", [128, J * D], F32R, kind="ExternalInput")
    v_in = nc.dram_tensor("vi", [128, J * VA], F32R, kind="ExternalInput")
    iota = nc.dram_tensor("io", [128, R], F32, kind="ExternalInput")
    thr = nc.dram_tensor("th", [128, J], F32, kind="ExternalInput")
    rdT = nc.dram_tensor("rdT", [D + 128, R], F32, kind="ExternalOutput")
    NN = D // 128  # 8 output feature chunks
    NQ = J // JQ   # 4 quarters
    with TileContext(nc) as tc:
        with tc.tile_pool(name="cst", bufs=1) as cst, \
             tc.tile_pool(name="kp", bufs=3) as kp, \
             tc.tile_pool(name="sp", bufs=2, space="PSUM") as sp, \
             tc.tile_pool(name="ep", bufs=3) as ep, \
             tc.tile_pool(name="pp", bufs=2 * JQ) as ppool, \
             tc.tile_pool(name="vp", bufs=2 * JQ) as vp, \
             tc.tile_pool(name="p2", bufs=6, space="PSUM") as p2, \
             tc.tile_pool(name="ac", bufs=NN + 1) as ac, \
             tc.tile_pool(name="no", bufs=5) as no:
            qt = cst.tile([128, KC * R], F32R)
            nc.sync.dma_start(qt[:, 0:R], qt_in[:, 0:R])
            io = cst.tile([128, R], F32)
            nc.sync.dma_start(io[:], iota[:])
            th = cst.tile([128, J], F32)
            nc.sync.dma_start(th[:], thr[:])
            # ---- per quarter: scores+exp+mask for 8 j-tiles, then PV ----
            pts = {}
            accs = {}
            for q in range(NQ):
                for jj in range(JQ):
                    j = q * JQ + jj
                    kt = kp.tile([128, D], F32R, name=f"kt{j}", tag="kt")
                    nc.sync.dma_start(kt[:], kt_in[:, j * D : (j + 1) * D])
                    if j == 0:
                        # remaining q chunks ride behind the first key tile so the
                        # first matmul only waits for ~1MB, not the whole q load
                        for k in range(1, KC):
                            nc.sync.dma_start(
                                qt[:, k * R : (k + 1) * R], qt_in[:, k * R : (k + 1) * R]
                            )
                    ps_ = sp.tile([128, R], F32, name=f"s{j}", tag="s")
                    for k in range(KC):
                        nc.tensor.matmul(
                            ps_[:],
                            kt[:, k * 128 : (k + 1) * 128],
                            qt[:, k * R : (k + 1) * R],
                            start=(k == 0), stop=(k == KC - 1),
                        )
                    ex = ep.tile([128, R], F32, name=f"e{j}", tag="e")
                    nc.scalar.activation(ex[:], ps_[:], mybir.ActivationFunctionType.Exp)
                    pt = ppool.tile([128, R], F32R, name=f"pt{j}", tag="pt")
                    nc.vector.scalar_tensor_tensor(
                        pt[:], io[:], th[:, j : j + 1], ex[:],
                        op0=mybir.AluOpType.is_le, op1=mybir.AluOpType.mult,
                    )
                    pts[j] = pt
                vts = []
                for jj in range(JQ):
                    j = q * JQ + jj
                    vt = vp.tile([128, VA], F32R, name=f"vt{j}", tag="vt")
                    nc.sync.dma_start(vt[:], v_in[:, j * VA : (j + 1) * VA])
                    vts.append(vt)
                for n in range(NN + 1):  # 8 feature chunks + (ones, pad) chunk
                    c0, w = (n * 128, 128) if n < NN else (D, 2)
                    pz = p2.tile([128, R], F32, name=f"pv{q}_{n}", tag="pv")
                    for jj in range(JQ):
                        j = q * JQ + jj
                        nc.tensor.matmul(
                            pz[:w, :],
                            vts[jj][:, c0 : c0 + w],
                            pts[j][:],
                            start=(jj == 0), stop=(jj == JQ - 1),
                        )
                    if q == 0:
                        a_ = ac.tile([128, R], F32, name=f"acc{n}", tag="ac")
                        accs[n] = a_
                        nc.vector.tensor_copy(a_[:w, :], pz[:w, :])
                    else:
                        a_ = accs[n]
                        nc.vector.tensor_add(a_[:w, :], a_[:w, :], pz[:w, :])
            # ---- ship unnormalized read^T + denominator row; host divides ----
            for n in range(NN):
                nc.sync.dma_start(rdT[n * 128 : (n + 1) * 128, :], accs[n][:])
            nc.sync.dma_start(rdT[D : D + 2, :], accs[NN][:2, :])
    nc.finalize()
    return nc


def _chunk_rows(a, nchunks):
    # [nchunks*128, C] -> [128, nchunks*C] with [p, k*C+c] = a[128k+p, c]
    n, c = a.shape
    assert n == nchunks * 128
    return np.ascontiguousarray(
        a.reshape(nchunks, 128, c).transpose(1, 0, 2).reshape(128, nchunks * c)
    )


def kernel(x, Wk, bk, Wq, bq, Wv, bv):
    x = np.asarray(x, dtype=np.float32)
    Wk = np.asarray(Wk, dtype=np.float32)
    Wq = np.asarray(Wq, dtype=np.float32)
    Wv = np.asarray(Wv, dtype=np.float32)
    bk = np.asarray(bk, dtype=np.float32)
    bq = np.asarray(bq, dtype=np.float32)
    bv = np.asarray(bv, dtype=np.float32)

    sc = np.float32(1.0 / np.sqrt(D))
    if "p1" not in _cache:
        _cache["p1"] = _build_phase1()
    if "p2" not in _cache:
        _cache["p2"] = _build_phase2()

    wq_in = _chunk_rows(Wq * sc, KC)
    wk_in = _chunk_rows(Wk, KC)
    wv_in = _chunk_rows(Wv, KC)
    in_maps1 = []
    for c in range(NCORES):
        xs = x[c * R : (c + 1) * R]
        xsT_in = _chunk_rows(np.ascontiguousarray(xs.T), KC)
        in_maps1.append({"xsT": xsT_in, "wq": wq_in, "wk": wk_in, "wv": wv_in})
    res1 = run_bass_kernel_spmd(_cache["p1"], in_maps1, list(range(NCORES))).results

    bq_s = (bq * sc)[None, :]
    qs = [res1[c]["q"] + bq_s for c in range(NCORES)]
    k_g = np.concatenate([res1[c]["k"] for c in range(NCORES)], axis=0) + bk[None, :]
    v_g = np.concatenate([res1[c]["v"] for c in range(NCORES)], axis=0) + bv[None, :]
    kT_g = np.ascontiguousarray(k_g.T)  # [D, S]
    v_aug = np.concatenate(
        [v_g, np.ones((S, 1), np.float32), np.zeros((S, 1), np.float32)], axis=1
    )

    # kt layout: [p, j, k, c] = kT_g[128k+p, 128j+c]
    kt_in = np.ascontiguousarray(
        kT_g.reshape(KC, 128, J, 128).transpose(1, 2, 0, 3).reshape(128, J * D)
    )
    v_in = _chunk_rows(v_aug, J)
    io_in = np.ascontiguousarray(
        np.broadcast_to(np.arange(R, dtype=np.float32), (128, R))
    )
    p_idx = np.arange(128, dtype=np.float32)[:, None]
    j_idx = np.arange(J, dtype=np.float32)[None, :]
    in_maps2 = []
    for c in range(NCORES):
        thr_c = np.ascontiguousarray(128.0 * j_idx + p_idx - 512.0 * c).astype(np.float32)
        in_maps2.append({
            "qt": _chunk_rows(np.ascontiguousarray(qs[c].T), KC),
            "kt": kt_in,
            "vi": v_in,
            "io": io_in,
            "th": thr_c,
        })
    res2 = run_bass_kernel_spmd(_cache["p2"], in_maps2, list(range(NCORES))).results

    read = np.concatenate(
        [(res2[c]["rdT"][:D] / res2[c]["rdT"][D : D + 1]).T for c in range(NCORES)], axis=0
    )
    return np.concatenate([x, read], axis=1)



# revision 2
# speedup vs baseline: 1.9333x; 1.9333x over previous
import sys

sys.path.insert(0, "/opt/trn_rl_repo")

import numpy as np
import ml_dtypes
import concourse.bass as bass  # noqa: F401  (registers types)
from concourse import bacc
import concourse.mybir as mybir
from concourse.tile import TileContext
from concourse.bass_utils import run_bass_kernel_spmd

S = 4096          # sequence length
D = 1024          # model/key/value dim
NCORES = 8
R = S // NCORES   # 512 rows per core
KK = 4            # 256-deep contraction double-chunks (DoubleRow)
J = S // 128      # 32 key tiles
VA = D + 2        # V augmented with ones column (denominator) + zero pad
CH = VA // 3      # 342-wide PV output chunks (3 chunks of 342, one PSUM bank each)
WSC = np.float32(16.0)   # fp8 prescale for projection weights

F32 = mybir.dt.float32
BF16 = mybir.dt.bfloat16
E4 = mybir.dt.float8e4
DR = mybir.MatmulPerfMode.DoubleRow
E4NP = ml_dtypes.float8_e4m3fn

_cache = {}


def _build_phase1():
    """Per core: q/k/v = xs @ (16*W) for its 512-row x slice, all fp8 DoubleRow.

    Contraction over the 1024 input dims runs as 4 double-chunks of 256.
    Stationary = x^T chunks, moving = weights; three per-projection passes of
    8 PSUM tiles (4 row-chunks x 2 column halves) so each pass only waits on
    its own 1MB weight load. Outputs are 16x the true projection, fp8; the
    host rescales and adds biases.
      xsT [128, 4, 2, 512]: [p, kk, t, r] = x[r, 128*(2kk+t)+p]
      w*  [128, 4, 2, 1024]: [p, kk, t, f] = 16*W[128*(2kk+t)+p, f]
    """
    nc = bacc.Bacc(None, target_bir_lowering=False)
    xsT = nc.dram_tensor("xsT", [128, KK, 2, R], E4, kind="ExternalInput")
    wins = [nc.dram_tensor(n, [128, KK, 2, D], E4, kind="ExternalInput")
            for n in ("wq", "wk", "wv")]
    outs = [nc.dram_tensor(n, [R, D], E4, kind="ExternalOutput") for n in ("q", "k", "v")]
    with TileContext(nc) as tc:
        with tc.tile_pool(name="inp", bufs=1) as inp, \
             tc.tile_pool(name="ob", bufs=8) as ob, \
             tc.tile_pool(name="ps", bufs=8, space="PSUM") as ps:
            xt = inp.tile([128, KK, 2, R], E4)
            nc.sync.dma_start(xt[:], xsT[:])
            wts = [inp.tile([128, KK, 2, D], E4, name=f"w{w_i}") for w_i in range(3)]
            for w_i in range(3):
                for kk in range(KK):
                    nc.sync.dma_start(wts[w_i][:, kk], wins[w_i][:, kk])
            for w_i in range(3):
                pz = [ps.tile([128, 512], F32, name=f"p{w_i}_{n2}", tag="ps")
                      for n2 in range(8)]
                for kk in range(KK):
                    for i in range(R // 128):
                        lhsT = xt[:, kk, :, i * 128 : (i + 1) * 128]
                        for h in range(2):
                            nc.tensor.matmul(
                                pz[i * 2 + h][:],
                                lhsT,
                                wts[w_i][:, kk, :, h * 512 : (h + 1) * 512],
                                start=(kk == 0), stop=(kk == KK - 1),
                                perf_mode=DR,
                            )
                for i in range(R // 128):
                    for h in range(2):
                        o = ob.tile([128, 512], E4, name=f"o{w_i}_{i}_{h}", tag="ob")
                        if (i * 2 + h) % 2 == 0:
                            nc.vector.tensor_copy(o[:], pz[i * 2 + h][:])
                        else:
                            nc.scalar.copy(o[:], pz[i * 2 + h][:])
                        nc.sync.dma_start(
                            outs[w_i][i * 128 : (i + 1) * 128, h * 512 : (h + 1) * 512], o[:]
                        )
    nc.finalize()
    return nc


def _build_phase2():
    """Per core: anti-causal attention for its 512 query rows vs all 4096 keys.

    Dense uniform program (SPMD: same code all cores; masking is data-driven
    via th). Scores computed transposed (keys on partitions) in fp8 DoubleRow,
    exp via scalar engine (softmax 1/sqrt(D) folded into activation scale),
    mask+fp8-quantize fused in one vector scalar_tensor_tensor per tile.
    PV flipped: out[queries, features], stationary = P^T chunks, moving = V,
    accumulated over key-pair tiles in PSUM (descending so vi DMA streams).
      qt [128, 4, 2, 512]: [p,kk,t,q] = qT[128*(2kk+t)+p, q]
      kt [128, 32, 4, 2, 128]: [p,j,kk,t,c] = kT[128*(2kk+t)+p, 128j+c]
      vi [128, 32, 1026]: [p,j,c] = v_aug[128j+p, c]
      io [128, 512]: iota cols; th [128, 32]: 128j+p-512*core
    Output rd [512, 1026] bf16 = unnormalized read + denominator column.
    """
    nc = bacc.Bacc(None, target_bir_lowering=False)
    qt_in = nc.dram_tensor("qt", [128, KK, 2, R], E4, kind="ExternalInput")
    kt_in = nc.dram_tensor("kt", [128, J, KK, 2, 128], E4, kind="ExternalInput")
    v_in = nc.dram_tensor("vi", [128, J, VA], E4, kind="ExternalInput")
    iota = nc.dram_tensor("io", [128, R], F32, kind="ExternalInput")
    thr = nc.dram_tensor("th", [128, J], F32, kind="ExternalInput")
    rd = nc.dram_tensor("rd", [R, VA], BF16, kind="ExternalOutput")
    with TileContext(nc) as tc:
        with tc.tile_pool(name="cst", bufs=1) as cst, \
             tc.tile_pool(name="sp", bufs=2, space="PSUM") as sp, \
             tc.tile_pool(name="ep", bufs=3) as ep, \
             tc.tile_pool(name="p2", bufs=2, space="PSUM") as p2, \
             tc.tile_pool(name="no", bufs=3) as no:
            io = cst.tile([128, R], F32)
            nc.sync.dma_start(io[:], iota[:])
            th = cst.tile([128, J], F32)
            nc.sync.dma_start(th[:], thr[:])
            qt = cst.tile([128, KK, 2, R], E4)
            nc.sync.dma_start(qt[:], qt_in[:])
            kt = cst.tile([128, J, KK, 2, 128], E4)
            for a in range(4):
                nc.sync.dma_start(kt[:, a * 8 : (a + 1) * 8], kt_in[:, a * 8 : (a + 1) * 8])
            vt = cst.tile([128, J, VA], E4)
            for a in range(3, -1, -1):
                nc.sync.dma_start(vt[:, a * 8 : (a + 1) * 8], v_in[:, a * 8 : (a + 1) * 8])
            pt = cst.tile([128, J, R], E4)
            # ---- scores: S^T[key, q] = K^T-chunks x Q, exp, mask -> fp8 P ----
            for j in range(J):
                ps_ = sp.tile([128, R], F32, name=f"s{j}", tag="s")
                for kk in range(KK):
                    nc.tensor.matmul(
                        ps_[:],
                        kt[:, j, kk],
                        qt[:, kk],
                        start=(kk == 0), stop=(kk == KK - 1),
                        perf_mode=DR,
                    )
                ex = ep.tile([128, R], F32, name=f"e{j}", tag="e")
                nc.scalar.activation(ex[:], ps_[:], mybir.ActivationFunctionType.Exp,
                                     scale=float(1.0 / np.sqrt(D)))
                nc.vector.scalar_tensor_tensor(
                    pt[:, j], io[:], th[:, j : j + 1], ex[:],
                    op0=mybir.AluOpType.is_le, op1=mybir.AluOpType.mult,
                )
            # ---- PV: read[q, f] accumulated over key pairs (descending) ----
            for qc in range(4):
                pz = [p2.tile([128, CH], F32, name=f"pv{qc}_{ch}", tag=f"ch{ch}")
                      for ch in range(3)]
                for m in range(J // 2 - 1, -1, -1):
                    lhsT = pt[:, 2 * m : 2 * m + 2, qc * 128 : (qc + 1) * 128]
                    for ch in range(3):
                        nc.tensor.matmul(
                            pz[ch][:],
                            lhsT,
                            vt[:, 2 * m : 2 * m + 2, ch * CH : (ch + 1) * CH],
                            start=(m == J // 2 - 1), stop=(m == 0),
                            perf_mode=DR,
                        )
                o = no.tile([128, VA], BF16, name=f"rd{qc}", tag="rd")
                nc.vector.tensor_copy(o[:, 0:CH], pz[0][:])
                nc.scalar.copy(o[:, CH : 2 * CH], pz[1][:])
                nc.vector.tensor_copy(o[:, 2 * CH : VA], pz[2][:])
                nc.sync.dma_start(rd[qc * 128 : (qc + 1) * 128, :], o[:])
    nc.finalize()
    return nc


def _dr_layout(aT):
    # [1024, C] (contraction-major) -> [128, 4, 2, C] DoubleRow layout
    c = aT.shape[1]
    return np.ascontiguousarray(aT.reshape(KK, 2, 128, c).transpose(2, 0, 1, 3))


def prep_phase1(x, Wq, Wk, Wv):
    xq = x.astype(E4NP)
    w_ins = [np.ascontiguousarray(
        _dr_layout((W * WSC).astype(E4NP).reshape(D, D))) for W in (Wq, Wk, Wv)]
    in_maps = []
    for c in range(NCORES):
        xsT = _dr_layout(np.ascontiguousarray(xq[c * R : (c + 1) * R].T))
        in_maps.append({"xsT": xsT, "wq": w_ins[0], "wk": w_ins[1], "wv": w_ins[2]})
    return in_maps


def prep_phase2(res1, bq, bk, bv):
    inv = np.float32(1.0 / WSC)
    q_g = np.concatenate([res1[c]["q"].astype(np.float32) for c in range(NCORES)]) * inv + bq
    k_g = np.concatenate([res1[c]["k"].astype(np.float32) for c in range(NCORES)]) * inv + bk
    v_g = np.concatenate([res1[c]["v"].astype(np.float32) for c in range(NCORES)]) * inv + bv
    kT = np.ascontiguousarray(k_g.T.astype(E4NP))
    kt_in = np.ascontiguousarray(
        kT.reshape(KK, 2, 128, J, 128).transpose(2, 3, 0, 1, 4))
    v_aug = np.concatenate(
        [v_g, np.ones((S, 1), np.float32), np.zeros((S, 1), np.float32)], axis=1)
    v_in = np.ascontiguousarray(
        v_aug.astype(E4NP).reshape(J, 128, VA).transpose(1, 0, 2))
    io_in = np.ascontiguousarray(
        np.broadcast_to(np.arange(R, dtype=np.float32), (128, R)))
    p_idx = np.arange(128, dtype=np.float32)[:, None]
    j_idx = np.arange(J, dtype=np.float32)[None, :]
    qT8 = q_g.T.astype(E4NP)
    in_maps = []
    for c in range(NCORES):
        qt = _dr_layout(np.ascontiguousarray(qT8[:, c * R : (c + 1) * R]))
        thr_c = np.ascontiguousarray(128.0 * j_idx + p_idx - 512.0 * c).astype(np.float32)
        in_maps.append({"qt": qt, "kt": kt_in, "vi": v_in, "io": io_in, "th": thr_c})
    return in_maps


def finish(x, res2):
    read = np.concatenate([
        (res2[c]["rd"][:, :D].astype(np.float32)
         / res2[c]["rd"][:, D : D + 1].astype(np.float32))
        for c in range(NCORES)], axis=0)
    return np.concatenate([x, read], axis=1)


def kernel(x, Wk, bk, Wq, bq, Wv, bv):
    x = np.asarray(x, dtype=np.float32)
    Wk = np.asarray(Wk, dtype=np.float32)
    Wq = np.asarray(Wq, dtype=np.float32)
    Wv = np.asarray(Wv, dtype=np.float32)
    bk = np.asarray(bk, dtype=np.float32)
    bq = np.asarray(bq, dtype=np.float32)
    bv = np.asarray(bv, dtype=np.float32)

    if "p1" not in _cache:
        _cache["p1"] = _build_phase1()
    if "p2" not in _cache:
        _cache["p2"] = _build_phase2()

    in_maps1 = prep_phase1(x, Wq, Wk, Wv)
    res1 = run_bass_kernel_spmd(_cache["p1"], in_maps1, list(range(NCORES))).results
    in_maps2 = prep_phase2(res1, bq, bk, bv)
    res2 = run_bass_kernel_spmd(_cache["p2"], in_maps2, list(range(NCORES))).results
    return finish(x, res2)
